# revision 13
# baseline (speedup 1.0000x reference)
"""Trainium2 Bass kernel for the R-BERT-style MoE routing head.

Computes, for x [B, H]:
    binary_logits = tanh(x) @ W_bin + b_bin          # [B, 2]
    route         = argmax(binary_logits, axis=1)    # ties -> 0
    logits        = (x @ W0 + b0) if route==0 else (x @ W1 + b1)   # [B, 30]

Data-parallel over 8 NeuronCores: x is sharded along the batch axis; the tiny
head weights are replicated. Per core, rows are processed in 128-row tiles:

  1. DMA the natural [128, 1024] x tile into SBUF (contiguous, full-BW).
  2. TensorE transpose-mode matmuls flip each [128, 128] chunk into PSUM so
     the contraction dim (H) lands on partitions.
  3. VectorE/ScalarE copy the transposed chunks to SBUF; ScalarE also applies
     tanh straight out of PSUM for the binary-head operand.
  4. TensorE accumulates W.T @ xT (experts, PSUM rows 0:60) and
     W_bin.T @ tanh(x)T (binary, PSUM rows 64:66) over the 8 K-chunks with
     the tiny weights stationary.
  5. Bias-add (per-partition tensor_scalar) + routing mask (bin1 > bin0),
     one more TensorE transpose back to row-major, then a predicated copy
     selects expert 0 vs 1 per row. One packed [128, 32] tile goes out.
"""

import numpy as np

B, H = 65536, 1024
NCORES = 8
BC = B // NCORES          # rows per core
PT = 128                  # rows per tile
KC = H // 128             # contraction chunks
NL = 30                   # labels per expert
NEXP = 2 * NL             # stacked expert outputs
NEXPP = 64                # expert outputs padded so psum rows 60:64 stay finite
BIN0 = 64                 # psum partition where binary logits start (32-aligned)
NROWS = BIN0 + 2          # psum rows: 60 expert + pad + 2 binary = 66
NOUT = NL + 2             # packed output cols: selected logits + binary

_CACHE = {}


def _build(bc):
    import concourse.bacc as bacc
    import concourse.tile as tile
    from concourse import mybir

    f32 = mybir.dt.float32
    nt = bc // PT

    nc = bacc.Bacc(
        "TRN2",
        target_bir_lowering=False,
        debug=False,
        enable_asserts=False,
        num_devices=NCORES,
    )

    xs = nc.dram_tensor("xs", [bc, H], f32, kind="ExternalInput")
    wexp = nc.dram_tensor("wexp", [128, KC, NEXPP], f32, kind="ExternalInput")
    wbin = nc.dram_tensor("wbin", [128, KC, 2], f32, kind="ExternalInput")
    bias = nc.dram_tensor("bias", [NROWS, 1], f32, kind="ExternalInput")
    ident = nc.dram_tensor("ident", [128, 128], f32, kind="ExternalInput")
    out = nc.dram_tensor("out", [bc, NOUT], f32, kind="ExternalOutput")

    with tile.TileContext(nc) as tc:
        with (
            tc.tile_pool(name="const", bufs=1) as cpool,
            tc.tile_pool(name="x", bufs=3) as xpool,
            tc.tile_pool(name="xt", bufs=2) as xtpool,
            tc.tile_pool(name="tt", bufs=2) as ttpool,
            tc.tile_pool(name="ot", bufs=2) as opool,
            tc.tile_pool(name="fin", bufs=3) as fpool,
            tc.tile_pool(name="msk", bufs=3) as mpool,
            tc.tile_pool(name="pxt", bufs=2, space="PSUM") as pxt,
            tc.tile_pool(name="po", bufs=2, space="PSUM") as pout,
            tc.tile_pool(name="pn", bufs=2, space="PSUM") as pnat,
        ):
            ident_sb = cpool.tile([128, 128], f32)
            nc.sync.dma_start(ident_sb[:], ident[:, :])
            wexp_sb = cpool.tile([128, KC, NEXPP], f32)
            nc.sync.dma_start(wexp_sb[:], wexp[:, :, :])
            wbin_sb = cpool.tile([128, KC, 2], f32)
            nc.sync.dma_start(wbin_sb[:], wbin[:, :, :])
            bias_sb = cpool.tile([NROWS, 1], f32)
            nc.sync.dma_start(bias_sb[:], bias[:, :])

            for t in range(nt):
                r0 = t * PT
                x_t = xpool.tile([PT, H], f32, tag="x")
                nc.sync.dma_start(x_t[:], xs[r0 : r0 + PT, :])

                psum_xT = pxt.tile([128, H], f32, tag="pxt")
                for k in range(KC):
                    sl = slice(k * 128, (k + 1) * 128)
                    nc.tensor.transpose(psum_xT[:, sl], x_t[:, sl], ident_sb[:])

                xT = xtpool.tile([128, H], f32, tag="xt")
                tT = ttpool.tile([128, H], f32, tag="tt")
                for k in range(KC):
                    sl = slice(k * 128, (k + 1) * 128)
                    # split the PSUM->SBUF evictions between DVE and ACT
                    if k < 6:
                        nc.vector.tensor_copy(xT[:, sl], psum_xT[:, sl])
                    else:
                        nc.scalar.copy(xT[:, sl], psum_xT[:, sl])
                    nc.scalar.activation(
                        tT[:, sl], psum_xT[:, sl], mybir.ActivationFunctionType.Tanh
                    )

                psum_o = pout.tile([BIN0 + 2, 128], f32, tag="po")
                for k in range(KC):
                    sl = slice(k * 128, (k + 1) * 128)
                    # expert + binary groups accumulate into disjoint partition
                    # ranges of one bank; has_written is per-element on HW, the
                    # sim's bank-granular group check is over-conservative.
                    nc.tensor.matmul(
                        psum_o[0:NEXPP, :],
                        wexp_sb[:, k, :],
                        xT[:, sl],
                        start=(k == 0),
                        stop=(k == KC - 1),
                        skip_group_check=True,
                    )
                    nc.tensor.matmul(
                        psum_o[BIN0 : BIN0 + 2, :],
                        wbin_sb[:, k, :],
                        tT[:, sl],
                        start=(k == 0),
                        stop=(k == KC - 1),
                        skip_group_check=True,
                    )

                # bias add (per-partition scalars)
                outT = opool.tile([NROWS, 128], f32, tag="ot")
                nc.vector.tensor_scalar_add(
                    outT[0:NEXPP, :], psum_o[0:NEXPP, :], bias_sb[0:NEXPP, 0:1]
                )
                nc.vector.tensor_scalar_add(
                    outT[BIN0 : BIN0 + 2, :],
                    psum_o[BIN0 : BIN0 + 2, :],
                    bias_sb[BIN0 : BIN0 + 2, 0:1],
                )

                # back to row-major: [66, 128] -> [128, 66]
                psum_n = pnat.tile([128, NROWS], f32, tag="pn")
                nc.tensor.transpose(
                    psum_n[:], outT[:], ident_sb[0:NROWS, 0:NROWS]
                )

                fin = fpool.tile([PT, NROWS], f32, tag="fin")
                nc.vector.tensor_copy(fin[:], psum_n[:])
                # routing mask: 1 where bin1 > bin0 (argmax ties pick 0);
                # walrus requires an integer mask dtype for CopyPredicated
                mask = mpool.tile([PT, 1], mybir.dt.uint8, tag="msk")
                nc.vector.tensor_tensor(
                    mask[:],
                    fin[:, BIN0 + 1 : BIN0 + 2],
                    fin[:, BIN0 : BIN0 + 1],
                    op=mybir.AluOpType.is_gt,
                )
                # route==1 rows take expert-1 logits
                nc.vector.copy_predicated(
                    fin[:, 0:NL],
                    mask[:].broadcast_to((PT, NL)),
                    fin[:, NL:NEXP],
                )
                # pack binary logits right after the selected logits
                nc.vector.tensor_copy(
                    fin[:, NL:NOUT], fin[:, BIN0 : BIN0 + 2]
                )
                nc.sync.dma_start(out[r0 : r0 + PT, :], fin[:, 0:NOUT])

    nc.compile()
    return nc


def _get_nc(bc):
    key = ("nc", bc)
    if key not in _CACHE:
        _CACHE[key] = _build(bc)
    return _CACHE[key]


def _host_inputs(W_bin, b_bin, W0, b0, W1, b1):
    f32 = np.float32
    wall = np.concatenate(
        [
            np.asarray(W0, f32),
            np.asarray(W1, f32),
            np.zeros((H, NEXPP - NEXP), f32),
        ],
        axis=1,
    )  # [H, 64] — last 4 cols are zero padding
    wexp = np.ascontiguousarray(
        wall.reshape(KC, 128, NEXPP).transpose(1, 0, 2)
    )  # [128, KC, 64]
    wbin = np.ascontiguousarray(
        np.asarray(W_bin, f32).reshape(KC, 128, 2).transpose(1, 0, 2)
    )  # [128, KC, 2]
    bias = np.zeros((NROWS, 1), f32)
    bias[0:NL, 0] = np.asarray(b0, f32)
    bias[NL:NEXP, 0] = np.asarray(b1, f32)
    bias[BIN0 : BIN0 + 2, 0] = np.asarray(b_bin, f32)
    ident = np.eye(128, dtype=f32)
    return wexp, wbin, bias, ident


def _run(x, W_bin, b_bin, W0, b0, W1, b1, **spmd_kwargs):
    from concourse.bass_utils import run_bass_kernel_spmd

    x = np.ascontiguousarray(np.asarray(x, np.float32))
    wexp, wbin, bias, ident = _host_inputs(W_bin, b_bin, W0, b0, W1, b1)

    nc = _get_nc(BC)
    in_maps = [
        {
            "xs": x[c * BC : (c + 1) * BC],
            "wexp": wexp,
            "wbin": wbin,
            "bias": bias,
            "ident": ident,
        }
        for c in range(NCORES)
    ]
    res = run_bass_kernel_spmd(
        nc, in_maps, core_ids=list(range(NCORES)), **spmd_kwargs
    )
    full = np.concatenate([res.results[c]["out"] for c in range(NCORES)], axis=0)
    binary_logits = np.ascontiguousarray(full[:, NL:NOUT])
    logits = np.ascontiguousarray(full[:, 0:NL])
    return (binary_logits, logits), res


def kernel(x, W_bin, b_bin, W0, b0, W1, b1):
    outs, _ = _run(x, W_bin, b_bin, W0, b0, W1, b1)
    return outs
